# revision 45
# baseline (speedup 1.0000x reference)
"""Fused transformer encoder layer (pre-LN, MHA + SiLU FFN) for Trainium2,
data-parallel over (batch, query-half) across 8 NeuronCores.

Contract: kernel(**inputs) takes the FULL unsharded inputs (numpy arrays, as
produced by the problem's setup_inputs) and returns the FULL [B, S, D] fp32
output. Sharding: core c handles batch b = c // 2 and query half h = c % 2
(1024 queries); each core computes K/V over the full 2048-token sequence of
its batch (keys/values are order-invariant under softmax, so each core gets
its x rotated to put its own queries first — one uniform SPMD program).

v2 design (ACT-exp-bound pipeline; ~110us of exp on ScalarE is the floor):
  - softmax denominator rides the AV matmul for free: V is packed per head
    as [V_h | ones-column] (M=33), so each PE pass over the exp weights
    produces ctx rows AND the denominator row — no separate denominator
    matmuls (the baseline re-streamed every attention weight through PE a
    second time just to sum it).
  - AV runs in fp8(e4m3) DoubleRow (two 128-key tiles contracted per pass,
    PE cost = N/2 cycles): exp writes fp8 directly, V is quantized
    host-side. Weight-quantization errors partially cancel between the
    numerator and the denominator of the softmax average.
  - attention is chunked as (2-head group) x (128-key tile) x (512
    queries): per chunk 2 scores matmuls (N=512, one PSUM bank each), one
    1024-element exp, and on odd chunks 2 DoubleRow AV matmuls. PE issues
    scores(chunk+1) before AV(chunk) so ACT's exp stream never waits.
  - non-attention work (proj, LN2, FFN, residuals) is emitted as deferred
    pieces drained one per chunk into PE/DVE idle slots under the exp
    stream, at token-tile/pair granularity so the post-stream tail is
    short. SiLU is computed as h*e/(1+e) with e=exp(h) on ACT and the
    rest on DVE, keeping ACT on the exp table set for the whole kernel
    (Exp and Silu live in different ACT table sets; a switch costs
    ~1.3us).
  - denominators are broadcast back over ctx rows with one pattern-matmul
    per (group, qc) into PSUM rows aligned with ctx (ctx rows {0-31,
    64-95}, den rows {32, 96}); proj weights are zero-padded at the den
    rows so ctx never needs repacking.
  - inputs arrive in 6 DMAs (4 x-chunks first, then two weight blobs) —
    per-DMA descriptor generation (~0.6us) makes many small weight DMAs
    the pre-phase critical path otherwise.
"""

import os
import sys

for _p in ("/opt/trn_rl_repo", "/root/.axon_site/_ro/trn_rl_repo"):
    if os.path.isdir(_p) and _p not in sys.path:
        sys.path.insert(0, _p)

import numpy as np
import ml_dtypes

import concourse.bass as bass
import concourse.tile as tile
from concourse import mybir
from concourse.bass_utils import run_bass_kernel_spmd
from concourse.vector_clock import ScopedClock

BF16 = ml_dtypes.bfloat16
E4M3 = ml_dtypes.float8_e4m3
F32 = mybir.dt.float32
BF = mybir.dt.bfloat16
FP8 = mybir.dt.float8e4

B, S, D = 4, 2048, 256
H, DH = 8, 32
DFF = 2 * D
SQ = S // 2          # queries per core
NQT = SQ // 128      # query token tiles per core (8)
NKT = S // 128       # key token tiles (16)
NTTP = NKT // 2      # key tile-pairs (8)
QCN = 2              # query chunks
QC = SQ // QCN       # 512 queries per chunk
NG = 4               # head groups of 2
EPS = 1e-5
AF = mybir.ActivationFunctionType
ALU = mybir.AluOpType
DR = mybir.MatmulPerfMode.DoubleRow

# ---- weight blob column offsets (bf16; 1-row tensors live in partition 0)
A_WQKV = 0                   # [128, 2, 768]
A_ID = A_WQKV + 1536         # [128, 128]
A_ONES = A_ID + 128          # row: [1, 1024]
A_ZROW = A_ONES + 1024       # row: [1, 128]
A_B1 = A_ZROW + 128          # row: [1, 512]
A_BPROJ = A_B1 + 512         # row: [1, 256]
A_B2 = A_BPROJ + 256         # row: [1, 256]
NA = A_B2 + 256
B_WPROJ = 0                  # [33, 8, 256] (row 32 zero-padded)
B_W1 = B_WPROJ + 2048        # [128, 2, 512]
B_W2 = B_W1 + 1024           # [128, 4, 256]
B_BV = B_W2 + 1024           # [128, 256]
B_RBCW = B_BV + 256          # [128, 128]
NB = B_RBCW + 128


def _merge_waits(nc):
    """The tile framework emits one wait per dependency edge; waits on the
    same monotonic semaphore with sem-ge-imm collapse to the max value."""
    for f in nc.m.functions:
        for bb in f.blocks:
            for ins in bb.instructions:
                si = getattr(ins, "sync_info", None)
                if si is None or len(si.on_wait) < 2:
                    continue
                best = {}
                rest = []
                for w in si.on_wait:
                    if w.wait_mode == "sem-ge-imm" and w.wait_reg is None:
                        key = (w.sync_type, w.id)
                        if key not in best or w.wait_value > best[key].wait_value:
                            best[key] = w
                    else:
                        rest.append(w)
                merged = list(best.values()) + rest
                if len(merged) < len(si.on_wait):
                    ins.sync_info = mybir.SyncInfo(
                        on_wait=merged, on_update=list(si.on_update))


def _split_excess_waits(nc, max_waits=1):
    """This walrus build lowers at most one sem wait per TPB instruction
    ("Too many sync wait commands" otherwise, matching bass's own
    inst_waits_full model). Move excess waits onto same-engine NoOps
    inserted directly before the instruction — the engine queue executes
    them in order, so the barrier semantics are unchanged."""
    cnt = 0
    for f in nc.m.functions:
        for bb in f.blocks:
            new = []
            changed = False
            for ins in bb.instructions:
                si = getattr(ins, "sync_info", None)
                waits = list(si.on_wait) if si is not None else []
                if len(waits) > max_waits:
                    for w in waits[:-max_waits]:
                        nop = mybir.InstNoOp(name=f"wsplit-{cnt}", ins=[], outs=[])
                        cnt += 1
                        nop.engine = ins.engine
                        nop.sync_info = mybir.SyncInfo(on_wait=[w], on_update=[])
                        new.append(nop)
                    ins.sync_info = mybir.SyncInfo(
                        on_wait=waits[-max_waits:], on_update=list(si.on_update))
                    changed = True
                new.append(ins)
            if changed:
                bb.instructions = new


def _patch_tile_drain():
    """walrus in this container rejects >~2 sem waits on the SP tail drain
    ("Too many sync wait commands"); emit one drain per clock proc instead."""

    def _drain_and_barrier(self, tick_clock, wait_clock):
        vclock = tick_clock.global_clock
        for proc in range(len(vclock)):
            t = vclock[proc]
            if t > 0:
                d = self.nc.sync.drain()
                part = ScopedClock()
                part.require_at_least(None, proc, t)
                wait_clock.add_sem_waits(d.ins, part)
        self.nc.all_engine_barrier()
        assert self.sems is not None
        popped = self.nc._tile_sem_poison_stack.pop()
        assert popped is self._sem_poison
        self.nc.clear_and_free_semaphores(list(self.sems.allocated().values()))
        self.nc.all_engine_barrier()

    tile.TileContext._drain_and_barrier = _drain_and_barrier


def _rsqrt_act(nc, pool, veps, nt, tag):
    """y = 1/sqrt(veps) as exp(-0.5*ln(veps)) on ACT — both Ln and Exp live
    in the 'natural_log_exp_and_others' table set, so this costs no table
    switch against the attention exp stream and no DVE Newton chain.
    (ACT's dedicated Rsqrt is banned for accuracy; Ln/Exp are <=2 ULP.)"""
    lnv = pool.tile([128, nt], F32, tag=f"rs_l{tag}")
    y = pool.tile([128, nt], F32, tag=f"rs_y{tag}")
    nc.scalar.activation(out=lnv, in_=veps, func=AF.Ln)
    nc.scalar.activation(out=y, in_=lnv, func=AF.Exp, scale=-0.5)
    return y


def _build_program():
    _patch_tile_drain()
    nc = bass.Bass()

    xkv = nc.dram_tensor("xkv", [S, D], F32, kind="ExternalInput")
    blobA = nc.dram_tensor("blobA", [128, NA], BF, kind="ExternalInput")
    blobB = nc.dram_tensor("blobB", [128, NB], BF, kind="ExternalInput")
    bqk = nc.dram_tensor("bqk", [128, 4], F32, kind="ExternalInput")
    out_d = nc.dram_tensor("out", [SQ, D], F32, kind="ExternalOutput")

    phases = int(os.environ.get("K_PHASES", "99"))
    reps = int(os.environ.get("K_REPS", "1"))

    with tile.TileContext(nc) as tc:
        with (
            tc.tile_pool(name="singles", bufs=1) as sg,
            tc.tile_pool(name="work", bufs=2) as wk,
            tc.tile_pool(name="attn", bufs=2) as atp,
            tc.tile_pool(name="psum", bufs=1, space="PSUM") as pp,
        ):
            # ---- persistent SBUF tensors
            x_sb = sg.tile([128, NKT, D], F32)          # full-seq x, token-major
            z1_sb = sg.tile([128, NKT, D], BF)
            zT = sg.tile([128, 2, S], BF)               # z1^T  [d, tok]
            qkT = sg.tile([128, 4, S], BF)              # Q^T (m 0-1), K^T (m 2-3)
            v8 = sg.tile([128, NTTP, 2, H, 48], FP8)    # [V_h | 1 | pad] per head
                                                        # (48B pitch: DoubleRow
                                                        # ldweights needs 16B-
                                                        # aligned steps/offsets)
            ctxP = sg.tile([33, H, SQ], BF)             # per head: rows 0-31 ctx,
                                                        # row 32 den leftover
            rec_b = sg.tile([33, 2, QC], BF)            # den recips at row 32
            x2_sb = sg.tile([128, NQT, D], F32)
            z2_sb = sg.tile([128, NQT, D], BF)
            z2T = sg.tile([128, 2, SQ], BF)
            hT = sg.tile([128, 4, SQ], BF)
            e_sb = sg.tile([128, 4, SQ], BF)            # exp(h) for silu
            u_sb = sg.tile([128, 4, SQ], BF)            # sigmoid(h) for silu
            out_sb = sg.tile([128, NQT, D], F32)

            wA = sg.tile([128, NA], BF)
            wB = sg.tile([128, NB], BF)
            bqk_sb = sg.tile([128, 4], F32)

            x_t = xkv.rearrange("(n p) d -> p n d", p=128)
            for nb in range(4):
                nc.sync.dma_start(out=x_sb[:, nb * 4:(nb + 1) * 4, :],
                                  in_=x_t[:, nb * 4:(nb + 1) * 4, :])
            nc.sync.dma_start(out=bqk_sb, in_=bqk[:])
            nc.sync.dma_start(out=wA, in_=blobA[:])
            nc.sync.dma_start(out=wB, in_=blobB[:])

            # weight blob views
            def wqkv(c, lo, hi):
                return wA[:, A_WQKV + c * 768 + lo:A_WQKV + c * 768 + hi]
            id_sb = wA[:, A_ID:A_ID + 128]
            def onesr(n):
                return wA[0:1, A_ONES:A_ONES + n]
            zrow_sb = wA[0:1, A_ZROW:A_ZROW + 128]
            def b1row(m):
                return wA[0:1, A_B1 + m * 128:A_B1 + (m + 1) * 128]
            bproj_sb = wA[0:1, A_BPROJ:A_BPROJ + 256]
            b2_sb = wA[0:1, A_B2:A_B2 + 256]
            def wproj(hh):
                return wB[0:33, B_WPROJ + hh * 256:B_WPROJ + (hh + 1) * 256]
            def w1(c, m):
                return wB[:, B_W1 + c * 512 + m * 128:B_W1 + c * 512 + (m + 1) * 128]
            def w2(c):
                return wB[:, B_W2 + c * 256:B_W2 + (c + 1) * 256]
            bv_sb = wB[:, B_BV:B_BV + 256]
            rbcw_sb = wB[:, B_RBCW:B_RBCW + 128]

            # one-time inits: ones columns of the V pack, zero rows of ctxP,
            # rec_b (the rbc matmul contracts over all 128 partitions; unused
            # rows hit zero weights but must be finite)
            nc.gpsimd.memset(v8[:, :, :, :, DH:DH + 1], 1.0)
            nc.gpsimd.memset(v8[:, :, :, :, DH + 1:], 0.0)
            nc.gpsimd.memset(rec_b[:], 0.0)

            # warm the ACT table set (ln+exp+identity) during the input DMAs
            warm = sg.tile([1, 8], F32)
            nc.vector.memset(warm[:], 1.0)
            nc.scalar.activation(out=warm[0:1, 4:8], in_=warm[0:1, 0:4], func=AF.Ln)
            nc.scalar.activation(out=warm[0:1, 0:4], in_=warm[0:1, 4:8], func=AF.Exp)

            gtag = [0]

            def gen_ps(shape, dtype=F32):
                t = pp.tile(shape, dtype, tag=f"gen{gtag[0] % 2}", name="gen")
                gtag[0] += 1
                return t

            out_t = out_d.rearrange("(n p) d -> p n d", p=128)

            def _one_rep():
                # ============ pre-attention, pipelined per 512-token block ====
                if phases < 1:
                    nc.sync.dma_start(out=out_t[:, 0, :], in_=x_sb[:, 0, :])
                    return
                for nb in range(4):
                    # LN1 stats for tiles 4nb..4nb+3, split ACT/DVE: ACT sums
                    # x and x^2 with its free-dim accumulator (Square and
                    # Identity are in every table set) for two tiles while
                    # DVE runs bn_stats for the other two — the pre-phase is
                    # otherwise serialized on DVE and gates K/V production
                    st6 = wk.tile([128, 2, 6], F32, tag="ln_st6")
                    mv = wk.tile([128, 4, 2], F32, tag="ln_mv")
                    sxa = wk.tile([128, 2, 2], F32, tag="ln_sx")
                    sq = wk.tile([128, 2, D], F32, tag="ln_sq")
                    veps = wk.tile([128, 4], F32, tag="ln_veps")
                    for k in (0, 1):
                        n = nb * 4 + k
                        nc.scalar.activation(out=sq[:, k, :], in_=x_sb[:, n, :],
                                             func=AF.Identity,
                                             accum_out=sxa[:, k, 0:1])
                        nc.scalar.activation(out=sq[:, k, :], in_=x_sb[:, n, :],
                                             func=AF.Square,
                                             accum_out=sxa[:, k, 1:2])
                    for k in (2, 3):
                        n = nb * 4 + k
                        nc.vector.bn_stats(out=st6[:, k - 2, :], in_=x_sb[:, n, :])
                        nc.vector.bn_aggr(out=mv[:, k, :], in_=st6[:, k - 2, :])
                    # mv[:, 0:2] from ACT sums: mu = sx/D, var = sx2/D - mu^2
                    nc.vector.tensor_scalar_mul(out=mv[:, 0:2, 0],
                                                in0=sxa[:, :, 0], scalar1=1.0 / D)
                    nc.vector.tensor_scalar_mul(out=mv[:, 0:2, 1],
                                                in0=sxa[:, :, 1], scalar1=1.0 / D)
                    vv = wk.tile([128, 2], F32, tag="ln_vv")
                    nc.vector.tensor_mul(out=vv, in0=mv[:, 0:2, 0], in1=mv[:, 0:2, 0])
                    nc.vector.tensor_tensor(mv[:, 0:2, 1], mv[:, 0:2, 1], vv,
                                            ALU.subtract)
                    nc.vector.tensor_scalar_add(out=veps, in0=mv[:, :, 1], scalar1=EPS)
                    y1 = _rsqrt_act(nc, wk, veps, 4, "1")
                    nmr = wk.tile([128, 4], F32, tag="ln_nmr")
                    nc.vector.scalar_tensor_tensor(out=nmr, in0=mv[:, :, 0],
                                                   scalar=-1.0, in1=y1,
                                                   op0=ALU.mult, op1=ALU.mult)
                    for k in range(4):
                        n = nb * 4 + k
                        nc.vector.tensor_scalar(
                            out=z1_sb[:, n, :], in0=x_sb[:, n, :],
                            scalar1=y1[:, k:k + 1], scalar2=nmr[:, k:k + 1],
                            op0=ALU.mult, op1=ALU.add)
                    if phases < 2:
                        continue
                    # transpose this block -> zT
                    for c in range(2):
                        tp = gen_ps([128, 512], BF)
                        for k in range(4):
                            n = nb * 4 + k
                            nc.tensor.transpose(
                                tp[:, k * 128:(k + 1) * 128],
                                z1_sb[:, n, c * 128:(c + 1) * 128], id_sb)
                        if nb == 0:
                            nc.scalar.copy(
                                out=zT[:, c, nb * 512:(nb + 1) * 512], in_=tp)
                        else:
                            nc.vector.tensor_copy(
                                out=zT[:, c, nb * 512:(nb + 1) * 512], in_=tp)
                    if phases < 3:
                        continue
                    # K^T for this block's keys; Q^T for blocks 0-1
                    qs = nb * 512
                    for m in (2, 3, 0, 1):
                        if m < 2 and nb != 0:
                            continue
                        ps = gen_ps([128, 512])
                        for c in range(2):
                            nc.tensor.matmul(
                                ps, lhsT=wqkv(c, m * 128, (m + 1) * 128),
                                rhs=zT[:, c, qs:qs + 512],
                                start=(c == 0), stop=(c == 1))
                        nc.vector.tensor_scalar(
                            out=qkT[:, m, qs:qs + 512], in0=ps,
                            scalar1=bqk_sb[:, m:m + 1], scalar2=None,
                            op0=ALU.add)
                    # V lags one block: K^T evacuations gate the stream
                    # start, V is first consumed a chunk into it — keep the
                    # DVE queue K-first within each block
                    vnb = nb - 1
                    vrange = (list(range(vnb * 4, vnb * 4 + 4)) if nb > 0 else []) \
                        + (list(range(12, 16)) if nb == 3 else [])
                    for tt in vrange:
                        ps = gen_ps([128, D])
                        for c in range(2):
                            nc.tensor.matmul(ps, lhsT=zT[:, c, tt * 128:(tt + 1) * 128],
                                             rhs=wqkv(c, 2 * D, 3 * D),
                                             start=(c == 0), stop=(c == 1))
                        nc.vector.tensor_tensor(
                            v8[:, tt // 2, tt % 2, :, 0:DH], ps, bv_sb, ALU.add)

                # ================= attention + deferred tail =================
                deferred = []

                def drain(k=1):
                    for _ in range(k):
                        if deferred:
                            deferred.pop(0)()

                def push(fn):
                    deferred.append(fn)

                ln2_t = [None, None]
                norm_t = [None]

                def proj_partial(g, qc):
                    # accumulate this head-group's proj contribution into
                    # x2 (f32, SBUF) as soon as its ctx rows are normalized;
                    # after the last group x2 = x + bproj + ctx @ wproj^T
                    if phases < 5:
                        return
                    for pr in range(2):
                        def pp_piece(g=g, qc=qc, pr=pr):
                            for n in range(4 * qc + 2 * pr, 4 * qc + 2 * pr + 2):
                                ps = gen_ps([128, D])
                                if g == 0:
                                    nc.tensor.matmul(ps, lhsT=onesr(128),
                                                     rhs=bproj_sb,
                                                     start=True, stop=False)
                                for h in range(2):
                                    hh = 2 * g + h
                                    nc.tensor.matmul(
                                        ps, lhsT=ctxP[:, hh, n * 128:(n + 1) * 128],
                                        rhs=wproj(hh),
                                        start=(g != 0 and h == 0),
                                        stop=(h == 1))
                                nc.vector.tensor_tensor(
                                    x2_sb[:, n, :], ps,
                                    x_sb[:, n, :] if g == 0 else x2_sb[:, n, :],
                                    ALU.add)
                        push(pp_piece)

                def post_qc(qc):
                    tail = qc == QCN - 1
                    if phases < 6:
                        return

                    # LN2 + transpose per pair of token tiles (256 queries)
                    for pr in range(2):
                        def ln2_a(qc=qc, pr=pr):
                            st2 = wk.tile([128, 2, 6], F32, tag="l2_st")
                            mv2 = wk.tile([128, 2, 2], F32, tag="l2_mv")
                            ln2_t[pr] = mv2
                            for i in range(2):
                                n = 4 * qc + 2 * pr + i
                                nc.vector.bn_stats(out=st2[:, i, :], in_=x2_sb[:, n, :])
                                nc.vector.bn_aggr(out=mv2[:, i, :], in_=st2[:, i, :])
                        def ln2_b(qc=qc, pr=pr):
                            mv2 = ln2_t[pr]
                            veps2 = wk.tile([128, 2], F32, tag="l2_veps")
                            nc.vector.tensor_scalar_add(out=veps2, in0=mv2[:, :, 1],
                                                        scalar1=EPS)
                            y2 = _rsqrt_act(nc, wk, veps2, 2, "2")
                            nmr2 = wk.tile([128, 2], F32, tag="l2_nmr")
                            nc.vector.scalar_tensor_tensor(
                                out=nmr2, in0=mv2[:, :, 0], scalar=-1.0, in1=y2,
                                op0=ALU.mult, op1=ALU.mult)
                            for i in range(2):
                                n = 4 * qc + 2 * pr + i
                                if tail:
                                    nc.scalar.activation(
                                        out=z2_sb[:, n, :], in_=x2_sb[:, n, :],
                                        func=AF.Identity, bias=nmr2[:, i:i + 1],
                                        scale=y2[:, i:i + 1])
                                else:
                                    nc.vector.tensor_scalar(
                                        out=z2_sb[:, n, :], in0=x2_sb[:, n, :],
                                        scalar1=y2[:, i:i + 1], scalar2=nmr2[:, i:i + 1],
                                        op0=ALU.mult, op1=ALU.add)
                        def z2t(qc=qc, pr=pr):
                            qs0 = qc * QC + pr * 256
                            for c in range(2):
                                tp = gen_ps([128, 256], BF)
                                for i in range(2):
                                    n = 4 * qc + 2 * pr + i
                                    nc.tensor.transpose(
                                        tp[:, i * 128:(i + 1) * 128],
                                        z2_sb[:, n, c * 128:(c + 1) * 128], id_sb)
                                if tail:
                                    nc.scalar.copy(
                                        out=z2T[:, c, qs0:qs0 + 256], in_=tp)
                                else:
                                    nc.vector.tensor_copy(
                                        out=z2T[:, c, qs0:qs0 + 256], in_=tp)
                        push(ln2_a)
                        push(ln2_b)
                        push(z2t)
                    if phases < 7:
                        return

                    # FFN1 + silu via exp table: h*e/(1+e), e on ACT, rest DVE
                    for m in range(4):
                        def ffn1(m=m, qc=qc):
                            qs = qc * QC
                            ps = gen_ps([128, QC])
                            nc.tensor.matmul(ps, lhsT=b1row(m), rhs=onesr(QC),
                                             start=True, stop=False)
                            for c in range(2):
                                nc.tensor.matmul(
                                    ps, lhsT=w1(c, m), rhs=z2T[:, c, qs:qs + QC],
                                    start=False, stop=(c == 1))
                            if tail:
                                # exp stream is over: one table swap to the
                                # native Silu beats the DVE sigmoid chain
                                nc.scalar.activation(out=hT[:, m, qs:qs + QC],
                                                     in_=ps, func=AF.Silu)
                            else:
                                e = e_sb[:, m, qs:qs + QC]
                                u = u_sb[:, m, qs:qs + QC]
                                nc.scalar.activation(out=e, in_=ps, func=AF.Exp)
                                with nc.allow_low_precision(reason="sigmoid in bf16"):
                                    nc.vector.tensor_scalar_add(out=u, in0=e, scalar1=1.0)
                                    nc.vector.reciprocal(out=u, in_=u)
                                nc.vector.tensor_mul(out=u, in0=e, in1=u)
                                nc.vector.tensor_mul(out=hT[:, m, qs:qs + QC],
                                                     in0=ps, in1=u)
                        push(ffn1)
                    if phases < 8:
                        return

                    # FFN2 + residual, out DMA per tile
                    for n in range(4 * qc, 4 * qc + 4):
                        def ffn2(n=n):
                            ps = gen_ps([128, D])
                            nc.tensor.matmul(ps, lhsT=onesr(128), rhs=b2_sb,
                                             start=True, stop=False)
                            for c in range(4):
                                nc.tensor.matmul(ps, lhsT=hT[:, c, n * 128:(n + 1) * 128],
                                                 rhs=w2(c), start=False, stop=(c == 3))
                            nc.vector.tensor_tensor(out_sb[:, n, :], ps,
                                                    x2_sb[:, n, :], ALU.add)
                            nc.sync.dma_start(out=out_t[:, n, :],
                                              in_=out_sb[:, n, :])
                        push(ffn2)

                # Q^T for qc1 is not needed until halfway through the
                # stream: emit it as early deferred pieces
                if phases >= 3:
                    for m in (0, 1):
                        def qlate(m=m):
                            ps = gen_ps([128, 512])
                            for c in range(2):
                                nc.tensor.matmul(
                                    ps, lhsT=wqkv(c, m * 128, (m + 1) * 128),
                                    rhs=zT[:, c, 512:1024],
                                    start=(c == 0), stop=(c == 1))
                            nc.vector.tensor_scalar(
                                out=qkT[:, m, 512:1024], in0=ps,
                                scalar1=bqk_sb[:, m:m + 1], scalar2=None,
                                op0=ALU.add)
                        push(qlate)

                if phases < 4:
                    drain(999)
                    return

                # chunk list: (qc, g, tt) — 2 heads, 128 keys, 512 queries
                chunks = [(qc, g, tt)
                          for qc in range(QCN) for g in range(NG)
                          for tt in range(NKT)]
                sc_tiles = {}
                ctx_tiles = {}
                at_cur = [None]

                def emit_scores(idx):
                    qc, g, tt = chunks[idx]
                    sc = pp.tile([128, 2, QC], F32, tag=f"sc{idx % 2}", name="sc")
                    sc_tiles[idx] = sc
                    if tt == 0:
                        ctx = pp.tile([33, 2, QC], F32, tag="ctx", name="ctx")
                        ctx_tiles[(qc, g)] = ctx
                    for h in range(2):
                        hh = 2 * g + h
                        band = 32 * (hh % 4)
                        nc.tensor.matmul(
                            sc[:, h, :],
                            lhsT=qkT[band:band + 32, 2 + hh // 4,
                                     tt * 128:(tt + 1) * 128],
                            rhs=qkT[band:band + 32, hh // 4,
                                    qc * QC:(qc + 1) * QC],
                            start=True, stop=True,
                            tile_position=(band, 0))

                emit_scores(0)
                for idx, (qc, g, tt) in enumerate(chunks):
                    sc = sc_tiles.pop(idx)
                    if tt % 2 == 0:
                        at_cur[0] = atp.tile([128, 2, 2, QC], FP8, name="at")
                    at = at_cur[0]
                    nc.scalar.activation(out=at[:, tt % 2, :, :], in_=sc, func=AF.Exp)
                    if idx + 1 < len(chunks):
                        emit_scores(idx + 1)
                    if tt % 2 == 1:
                        ctx = ctx_tiles[(qc, g)]
                        for h in range(2):
                            nc.tensor.matmul(
                                ctx[0:33, h, :],
                                lhsT=v8[:, tt // 2, :, 2 * g + h, 0:DH + 1],
                                rhs=at[:, :, h, :],
                                start=(tt == 1), stop=(tt == NKT - 1),
                                perf_mode=DR)
                        if tt == NKT - 1:
                            # normalize as deferred pieces: the rbc matmuls
                            # must not enter the PE queue until the DVE
                            # reciprocal is done, or they stall the next
                            # block's scores (and with them the exp stream)
                            ctx_ps = ctx_tiles.pop((qc, g))
                            qs = qc * QC

                            tail_n = qc == QCN - 1 and g == NG - 1

                            def n1(ctx_ps=ctx_ps, g=g, tail_n=tail_n):
                                ctxU = wk.tile([33, 2, QC], BF, tag="ctxu")
                                norm_t[0] = ctxU
                                if tail_n:
                                    # ACT is idle after the last exp; keep the
                                    # DVE serial chain short in the tail
                                    nc.scalar.copy(out=ctxU, in_=ctx_ps)
                                else:
                                    nc.vector.tensor_copy(out=ctxU, in_=ctx_ps)
                                with nc.allow_low_precision(reason="1/den bf16"):
                                    nc.vector.reciprocal(out=rec_b[32:33, :, :],
                                                         in_=ctxU[32:33, :, :])

                            def n2(qs=qs, g=g, h=0):
                                ctxU = norm_t[0]
                                rbc = gen_ps([33, QC])
                                nc.tensor.matmul(rbc, lhsT=rbcw_sb[0:33, 0:33],
                                                 rhs=rec_b[0:33, h, :],
                                                 start=True, stop=True)
                                nc.vector.tensor_tensor(
                                    ctxP[0:33, 2 * g + h, qs:qs + QC],
                                    ctxU[:, h, :], rbc, ALU.mult)

                            def n3(qs=qs, g=g):
                                n2(qs, g, 1)

                            push(n1)
                            # space out the PE-bearing pieces: n2's rbc
                            # matmul must not reach the PE queue before the
                            # reciprocal (DVE) it waits on has finished
                            push(lambda: None)
                            push(lambda: None)
                            push(n2)
                            push(n3)
                            proj_partial(g, qc)
                            if g == NG - 1:
                                post_qc(qc)
                    drain(1)
                drain(999)

            for _rep in range(reps):
                _one_rep()

    _merge_waits(nc)
    _split_excess_waits(nc)
    return nc


_PROGRAM = None
last_exec_time_ns = None


def _get_program():
    global _PROGRAM
    if _PROGRAM is None:
        _PROGRAM = _build_program()
    return _PROGRAM


def kernel(x, ln1_g, ln1_b, w_qkv, b_qkv, w_proj, b_proj,
           ln2_g, ln2_b, w1, b1, w2, b2):
    global last_exec_time_ns
    x = np.asarray(x, np.float32)
    ln1_g = np.asarray(ln1_g, np.float32)
    ln1_b = np.asarray(ln1_b, np.float32)
    w_qkv = np.asarray(w_qkv, np.float32)
    b_qkv = np.asarray(b_qkv, np.float32)
    w_proj = np.asarray(w_proj, np.float32)
    b_proj = np.asarray(b_proj, np.float32)
    ln2_g = np.asarray(ln2_g, np.float32)
    ln2_b = np.asarray(ln2_b, np.float32)
    w1 = np.asarray(w1, np.float32)
    b1 = np.asarray(b1, np.float32)
    w2 = np.asarray(w2, np.float32)
    b2 = np.asarray(b2, np.float32)

    scale = DH ** -0.5
    w_qkv_eff = w_qkv * ln1_g[None, :]
    b_qkv_eff = (b_qkv + w_qkv @ ln1_b).copy()
    w_qkv_eff[:D] *= scale
    b_qkv_eff[:D] *= scale
    w1_eff = w1 * ln2_g[None, :]
    b1_eff = b1 + w1 @ ln2_b

    def fmt_T(w):  # [out_f, in_d] -> [128, in_chunks * out_f] bf16
        o, d = w.shape
        return np.ascontiguousarray(
            w.T.reshape(d // 128, 128, o).transpose(1, 0, 2)).reshape(128, -1)

    # proj weights in the den-row-padded ctx layout: chunk h has the head's
    # input-feature rows at 0-31; row 32 (the denominator leftover) is zero.
    wprojT_pad = np.zeros((128, 8, D), np.float32)
    wpT = w_proj.T  # [in_feature, out]
    for hh in range(8):
        wprojT_pad[0:32, hh, :] = wpT[32 * hh:32 * hh + 32]

    rbcw = np.zeros((128, 128), np.float32)
    rbcw[32, 0:33] = 1.0

    blobA = np.zeros((128, NA), np.float32)
    blobA[:, A_WQKV:A_WQKV + 1536] = fmt_T(w_qkv_eff)
    blobA[:, A_ID:A_ID + 128] = np.eye(128)
    blobA[0, A_ONES:A_ONES + 1024] = 1.0
    blobA[0, A_B1:A_B1 + 512] = b1_eff
    blobA[0, A_BPROJ:A_BPROJ + 256] = b_proj
    blobA[0, A_B2:A_B2 + 256] = b2

    blobB = np.zeros((128, NB), np.float32)
    blobB[:, B_WPROJ:B_WPROJ + 2048] = wprojT_pad.reshape(128, -1)
    blobB[:, B_W1:B_W1 + 1024] = fmt_T(w1_eff)
    blobB[:, B_W2:B_W2 + 1024] = fmt_T(w2)
    blobB[:, B_BV:B_BV + 256] = np.broadcast_to(b_qkv_eff[512:768][None, :], (128, D))
    blobB[:, B_RBCW:B_RBCW + 128] = rbcw

    shared = {
        "blobA": blobA.astype(BF16),
        "blobB": blobB.astype(BF16),
        "bqk": np.ascontiguousarray(
            b_qkv_eff[0:512].reshape(4, 128).T).astype(np.float32),
    }

    in_maps = []
    for c in range(8):
        b, hh = divmod(c, 2)
        xr = np.concatenate([x[b, hh * SQ:(hh + 1) * SQ],
                             x[b, (1 - hh) * SQ:(2 - hh) * SQ]], axis=0)
        m = dict(shared)
        m["xkv"] = np.ascontiguousarray(xr)
        in_maps.append(m)

    trace = os.environ.get("BASS_KERNEL_TRACE") == "1"
    res = run_bass_kernel_spmd(_get_program(), in_maps,
                               core_ids=list(range(8)), trace=trace)
    last_exec_time_ns = res.exec_time_ns

    out = np.empty((B, S, D), np.float32)
    for c in range(8):
        b, hh = divmod(c, 2)
        out[b, hh * SQ:(hh + 1) * SQ] = res.results[c]["out"]
    return out
